# revision 3
# baseline (speedup 1.0000x reference)
"""Trainium2 Bass kernel: 16-head MHA with RoPE (B=4, N=2048, D=1024).

Sharding (8 cores): core c -> (batch b = c//2, head-group g = c%2 of 8 heads).
Each core computes its 8 heads' attention for one batch and a partial
projection output; the host sums the two partials per batch and adds b_proj.

Per-core pipeline (all layouts transposed so no on-device transposes needed):
  qkv^T = W^T x^T (fp32r matmuls, W stationary), RoPE fused into the
  PSUM->SBUF evacuation via scalar_tensor_tensor quarter-ops;
  scores S^T[k,q] = K^T.T Q^T per head-pair via 64-row PE tiling (2 heads
  concurrently); softmax without max-subtraction (scores are O(2) here),
  exp on ScalarE straight from PSUM into bf16 P^T tiles; A@V via a
  zero-padded stationary [v_even|1|0..|1|0..|v_odd] so each head's output
  lands on its own partition range and the ones-column yields the softmax
  denominators; normalization fused into the PSUM evacuation; projection
  contracts the pair-stacked O^T tiles (full 128 contraction) accumulating
  all 4 pairs in PSUM.
"""

import numpy as np

EMBED = 1024
NHEAD = 16
HD = 64
SCALE = HD ** -0.5
B = 4
N = 2048
NCORES = 8

_CACHE = {}


def _build_nc(niter=1):
    import concourse.bacc as bacc
    import concourse.mybir as mybir
    from concourse.tile import TileContext

    f32 = mybir.dt.float32
    f32r = mybir.dt.float32r
    bf16 = mybir.dt.bfloat16
    A = mybir.AluOpType
    Act = mybir.ActivationFunctionType

    nc = bacc.Bacc(None, target_bir_lowering=False)

    xT = nc.dram_tensor("xT", [EMBED, N], f32, kind="ExternalInput")
    wqk = nc.dram_tensor("wqk", [EMBED, 1024], f32, kind="ExternalInput")
    wv = nc.dram_tensor("wv", [EMBED, 512], f32, kind="ExternalInput")
    wp = nc.dram_tensor("wp", [512, 1024], f32, kind="ExternalInput")
    bqk = nc.dram_tensor("bqk", [128, 8], f32, kind="ExternalInput")
    bqkp = nc.dram_tensor("bqkp", [128, 8], f32, kind="ExternalInput")
    cos2 = nc.dram_tensor("cos2", [128, N], f32, kind="ExternalInput")
    sinS = nc.dram_tensor("sinS", [128, N], f32, kind="ExternalInput")
    bvo = nc.dram_tensor("bvo", [128, 512], f32, kind="ExternalInput")
    y = nc.dram_tensor("y", [N, 1024], f32, kind="ExternalOutput")

    with TileContext(nc) as tc:
        for _ in range(niter):
            _emit_iter(nc, tc, mybir, f32, f32r, bf16, A, Act,
                       xT, wqk, wv, wp, bqk, bqkp, cos2, sinS, bvo, y)

    nc.finalize()
    return nc


def _emit_iter(nc, tc, mybir, f32, f32r, bf16, A, Act,
               xT, wqk, wv, wp, bqk, bqkp, cos2, sinS, bvo, y):
    VBLK = 224  # per-pair block in the packed V tile

    with tc.tile_pool(name="persist", bufs=1) as Pp:
        qk_t = [Pp.tile([128, N], f32r, tag=f"qk{i}", name=f"qk{i}") for i in range(8)]
        va_t = [Pp.tile([128, 4 * VBLK], bf16, tag=f"va{i}", name=f"va{i}") for i in range(16)]
        oT_t = [Pp.tile([128, N], f32r, tag=f"o{j}", name=f"o{j}") for j in range(4)]

        # ---------------- phase 1: QKV + RoPE ----------------
        with (
            tc.tile_pool(name="qkv", bufs=1) as Pq,
            tc.tile_pool(name="ps_qk", bufs=3, space="PSUM") as Sqk,
            tc.tile_pool(name="ps_v", bufs=2, space="PSUM") as Sv,
        ):
            c2t = Pq.tile([128, N], f32, tag="cos2")
            sSt = Pq.tile([128, N], f32, tag="sinS")
            bqt = Pq.tile([128, 8], f32, tag="bqk")
            bqpt = Pq.tile([128, 8], f32, tag="bqkp")
            bvt = Pq.tile([128, 512], f32, tag="bvo")
            nc.sync.dma_start(out=c2t[:], in_=cos2[:, :])
            nc.sync.dma_start(out=sSt[:], in_=sinS[:, :])
            nc.sync.dma_start(out=bqt[:], in_=bqk[:, :])
            nc.sync.dma_start(out=bqpt[:], in_=bqkp[:, :])
            nc.sync.dma_start(out=bvt[:], in_=bvo[:, :])
            wvt = []
            for dk in range(8):
                w = Pq.tile([128, 512], f32r, tag=f"wv{dk}")
                nc.gpsimd.dma_start(out=w[:], in_=wv[128 * dk:128 * dk + 128, :])
                wvt.append(w)

            for half in range(2):
                hsl = slice(1024 * half, 1024 * half + 1024)
                xt = []
                for dk in range(8):
                    t = Pq.tile([128, 1024], f32r, tag=f"xt{dk}")
                    nc.gpsimd.dma_start(out=t[:], in_=xT[128 * dk:128 * dk + 128, hsl])
                    xt.append(t)

                # v (natural layout) for this half's 8 row-tiles
                for rt8 in range(8):
                    rt = 8 * half + rt8
                    pv = Sv.tile([128, 512], f32, tag="v")
                    for dk in range(8):
                        nc.tensor.matmul(
                            pv[:],
                            lhsT=xt[dk][:, 128 * rt8:128 * rt8 + 128],
                            rhs=wvt[dk][:],
                            start=(dk == 0), stop=(dk == 7))
                    va = va_t[rt]
                    nc.gpsimd.memset(va[:], 0)
                    vav = va[:].rearrange("p (j c) -> p j c", j=4, c=VBLK)
                    nc.vector.memset(vav[:, :, 64:65], 1.0)
                    nc.vector.memset(vav[:, :, 128:129], 1.0)
                    pvv = pv[:].rearrange("p (j s c) -> p j s c", j=4, s=2, c=64)
                    bvv = bvt[:].rearrange("p (j s c) -> p j s c", j=4, s=2, c=64)
                    nc.vector.tensor_add(vav[:, :, 0:64], pvv[:, :, 0, :], bvv[:, :, 0, :])
                    nc.vector.tensor_add(vav[:, :, 160:224], pvv[:, :, 1, :], bvv[:, :, 1, :])

                # q^T / k^T col-tiles with fused RoPE evacuation
                for ct in range(8):
                    wt = []
                    for dk in range(8):
                        t = Pq.tile([128, 128], f32r, tag=f"wq{dk}")
                        nc.gpsimd.dma_start(
                            out=t[:],
                            in_=wqk[128 * dk:128 * dk + 128, 128 * ct:128 * ct + 128])
                        wt.append(t)
                    pqk = Sqk.tile([128, 1024], f32, tag="qk")
                    for dk in range(8):
                        for qc in range(2):
                            nc.tensor.matmul(
                                pqk[:, 512 * qc:512 * qc + 512],
                                lhsT=wt[dk][:],
                                rhs=xt[dk][:, 512 * qc:512 * qc + 512],
                                start=(dk == 0), stop=(dk == 7))
                    dst = qk_t[ct][:, hsl]
                    scr = Pq.tile([128, 1024], f32, tag="ropescr")
                    nc.vector.scalar_tensor_tensor(
                        out=dst, in0=pqk[:], scalar=bqt[:, ct:ct + 1],
                        in1=c2t[:, hsl], op0=A.add, op1=A.mult)
                    for h2 in range(2):
                        b0 = 64 * h2
                        nc.vector.scalar_tensor_tensor(
                            out=scr[b0:b0 + 32, :], in0=pqk[b0 + 32:b0 + 64, :],
                            scalar=bqpt[b0:b0 + 32, ct:ct + 1],
                            in1=sSt[b0:b0 + 32, hsl], op0=A.add, op1=A.mult)
                        nc.vector.scalar_tensor_tensor(
                            out=scr[b0 + 32:b0 + 64, :], in0=pqk[b0:b0 + 32, :],
                            scalar=bqpt[b0 + 32:b0 + 64, ct:ct + 1],
                            in1=sSt[b0 + 32:b0 + 64, hsl], op0=A.add, op1=A.mult)
                    nc.vector.tensor_add(dst, dst, scr[:])

        # ---------------- phase 2: attention ----------------
        with (
            tc.tile_pool(name="attn", bufs=1) as Pa,
            tc.tile_pool(name="ps_s", bufs=2, space="PSUM") as Ss,
            tc.tile_pool(name="ps_av", bufs=2, space="PSUM") as Sav,
        ):
            recA = Pa.tile([1, 1024], f32, tag="recA")
            recB = Pa.tile([1, 1024], f32, tag="recB")
            rbcA = Pa.tile([128, 1024], f32, tag="rbcA")
            rbcB = Pa.tile([128, 1024], f32, tag="rbcB")

            for j in range(4):
                qT_, kT_ = qk_t[j], qk_t[4 + j]
                for qh in range(2):
                    qsl = slice(1024 * qh, 1024 * qh + 1024)
                    av = [Sav.tile([128, 1024], f32, tag="av", name=f"av{i}") for i in range(2)]
                    pT = [[None] * 16, [None] * 16]
                    for kc in range(16):
                        ks = slice(128 * kc, 128 * kc + 128)
                        sA = Ss.tile([128, 1024], f32, tag="s")
                        sB = Ss.tile([128, 1024], f32, tag="s")
                        for qc in range(2):
                            qq = slice(1024 * qh + 512 * qc, 1024 * qh + 512 * qc + 512)
                            nc.tensor.matmul(
                                sA[:, 512 * qc:512 * qc + 512],
                                lhsT=kT_[0:64, ks], rhs=qT_[0:64, qq],
                                tile_position=(0, 0), start=True, stop=True)
                            nc.tensor.matmul(
                                sB[:, 512 * qc:512 * qc + 512],
                                lhsT=kT_[64:128, ks], rhs=qT_[64:128, qq],
                                tile_position=(64, 0), start=True, stop=True)
                        pT[0][kc] = Pa.tile([128, 1024], bf16, tag="pA", bufs=4, name=f"pA{kc}")
                        pT[1][kc] = Pa.tile([128, 1024], bf16, tag="pB", bufs=4, name=f"pB{kc}")
                        nc.scalar.activation(pT[0][kc][:], sA[:], Act.Exp, scale=SCALE)
                        nc.scalar.activation(pT[1][kc][:], sB[:], Act.Exp, scale=SCALE)
                        if kc > 0:
                            _emit_av(nc, va_t, pT, av, j, kc - 1)
                    _emit_av(nc, va_t, pT, av, j, 15)
                    # normalize + evacuate
                    nc.vector.reciprocal(recA[0:1, :], av[0][64:65, :])
                    nc.vector.reciprocal(recB[0:1, :], av[1][32:33, :])
                    nc.gpsimd.partition_broadcast(rbcA[:, :], recA[0:1, :])
                    nc.gpsimd.partition_broadcast(rbcB[:, :], recB[0:1, :])
                    nc.vector.tensor_mul(oT_t[j][0:64, qsl], av[0][0:64, :], rbcA[0:64, :])
                    nc.vector.tensor_mul(oT_t[j][64:128, qsl], av[1][64:128, :], rbcB[64:128, :])

        # ---------------- phase 3: projection ----------------
        with (
            tc.tile_pool(name="proj", bufs=1) as Pj,
            tc.tile_pool(name="ps_y", bufs=2, space="PSUM") as Sy,
        ):
            wpt = []
            for j in range(4):
                t = Pj.tile([128, 1024], f32r, tag=f"wp{j}")
                nc.gpsimd.dma_start(out=t[:], in_=wp[128 * j:128 * j + 128, :])
                wpt.append(t)
            for rt in range(16):
                py = Sy.tile([128, 1024], f32, tag="y")
                for j in range(4):
                    for yc in range(2):
                        nc.tensor.matmul(
                            py[:, 512 * yc:512 * yc + 512],
                            lhsT=oT_t[j][:, 128 * rt:128 * rt + 128],
                            rhs=wpt[j][:, 512 * yc:512 * yc + 512],
                            start=(j == 0), stop=(j == 3))
                ysb = Pj.tile([128, 1024], f32, tag="ysb", bufs=2)
                nc.vector.tensor_copy(ysb[:], py[:])
                nc.sync.dma_start(out=y[128 * rt:128 * rt + 128, :], in_=ysb[:])


def _emit_av(nc, va_t, pT, av, j, kc):
    VBLK = 224
    for hi in range(2):
        lo = VBLK * j + (96 if hi else 0)
        for qc in range(2):
            nc.tensor.matmul(
                av[hi][:, 512 * qc:512 * qc + 512],
                lhsT=va_t[kc][:, lo:lo + 128],
                rhs=pT[hi][kc][:, 512 * qc:512 * qc + 512],
                start=(kc == 0), stop=(kc == 15))


def _shard_inputs(x, rope_cos, rope_sin, W_qkv, b_qkv, W_proj):
    cos2 = np.ascontiguousarray(
        np.concatenate([rope_cos.T, rope_cos.T], 0), dtype=np.float32)
    sT = rope_sin.T
    s64 = np.concatenate([-sT[:32], sT[32:]], 0)
    sinS = np.ascontiguousarray(np.concatenate([s64, s64], 0), dtype=np.float32)

    per_g = []
    for g in range(2):
        o = 512 * g
        wqk = np.ascontiguousarray(
            np.concatenate([W_qkv[:, o:o + 512], W_qkv[:, 1024 + o:1024 + o + 512]], 1))
        wv = np.ascontiguousarray(W_qkv[:, 2048 + o:2048 + o + 512])
        wp = np.ascontiguousarray(W_proj[o:o + 512, :])
        bqk_cat = np.concatenate([b_qkv[o:o + 512], b_qkv[1024 + o:1024 + o + 512]])
        bqk = np.ascontiguousarray(bqk_cat.reshape(8, 128).T)
        bqkp_cat = bqk_cat.reshape(16, 2, 32)[:, ::-1, :].reshape(1024)
        bqkp = np.ascontiguousarray(bqkp_cat.reshape(8, 128).T)
        bvo = np.ascontiguousarray(
            np.broadcast_to(b_qkv[2048 + o:2048 + o + 512], (128, 512)))
        per_g.append(dict(wqk=wqk, wv=wv, wp=wp, bqk=bqk, bqkp=bqkp, bvo=bvo))

    xTs = [np.ascontiguousarray(x[b].T) for b in range(B)]
    in_maps = []
    for c in range(NCORES):
        b, g = c // 2, c % 2
        m = dict(per_g[g])
        m["xT"] = xTs[b]
        m["cos2"] = cos2
        m["sinS"] = sinS
        in_maps.append(m)
    return in_maps


def _get_runner(niter=1):
    key = ("runner", niter)
    if key in _CACHE:
        return _CACHE[key]
    import jax
    from jax.sharding import Mesh, PartitionSpec
    from jax.experimental.shard_map import shard_map
    from concourse import bass2jax

    nc = _build_nc(niter)
    bass2jax.install_neuronx_cc_hook()

    import concourse.mybir as mybir
    partition_name = nc.partition_id_tensor.name if nc.partition_id_tensor else None
    in_names, out_names, out_avals, zero_outs = [], [], [], []
    for alloc in nc.m.functions[0].allocations:
        if not isinstance(alloc, mybir.MemoryLocationSet):
            continue
        name = alloc.memorylocations[0].name
        if alloc.kind == "ExternalInput":
            if name != partition_name:
                in_names.append(name)
        elif alloc.kind == "ExternalOutput":
            shape = tuple(alloc.tensor_shape)
            np_dtype = mybir.dt.np(alloc.dtype)
            out_names.append(name)
            out_avals.append(jax.core.ShapedArray(shape, np_dtype))
            zero_outs.append(np.zeros(shape, np_dtype))

    n_params = len(in_names)
    n_outs = len(out_names)
    all_in_names = list(in_names) + list(out_names)
    if partition_name is not None:
        all_in_names.append(partition_name)
    donate = tuple(range(n_params, n_params + n_outs))

    def _body(*args):
        operands = list(args)
        if partition_name is not None:
            operands.append(bass2jax.partition_id_tensor())
        outs = bass2jax._bass_exec_p.bind(
            *operands,
            out_avals=tuple(out_avals),
            in_names=tuple(all_in_names),
            out_names=tuple(out_names),
            lowering_input_output_aliases=(),
            sim_require_finite=True,
            sim_require_nnan=True,
            nc=nc,
        )
        return tuple(outs)

    devices = jax.devices()[:NCORES]
    mesh = Mesh(np.asarray(devices), ("core",))
    in_specs = (PartitionSpec("core"),) * (n_params + n_outs)
    out_specs = (PartitionSpec("core"),) * n_outs
    sharded = jax.jit(
        shard_map(_body, mesh=mesh, in_specs=in_specs, out_specs=out_specs,
                  check_rep=False),
        donate_argnums=donate, keep_unused=True)

    runner = dict(sharded=sharded, in_names=in_names, out_names=out_names,
                  out_avals=out_avals, zero_outs=zero_outs, nc=nc)
    _CACHE[key] = runner
    return runner


def _run_spmd(in_maps, niter=1):
    r = _get_runner(niter)
    concat_in = [
        np.concatenate([np.asarray(in_maps[c][name]) for c in range(NCORES)], axis=0)
        for name in r["in_names"]
    ]
    concat_zeros = [
        np.zeros((NCORES * z.shape[0], *z.shape[1:]), z.dtype) for z in r["zero_outs"]
    ]
    out_arrs = r["sharded"](*concat_in, *concat_zeros)
    outs = []
    for c in range(NCORES):
        m = {}
        for i, name in enumerate(r["out_names"]):
            shape = r["out_avals"][i].shape
            m[name] = np.asarray(out_arrs[i]).reshape(NCORES, *shape)[c]
        outs.append(m)
    return outs


def kernel(**inputs):
    x = np.asarray(inputs["x"], np.float32)
    rope_cos = np.asarray(inputs["rope_cos"], np.float32)
    rope_sin = np.asarray(inputs["rope_sin"], np.float32)
    W_qkv = np.asarray(inputs["W_qkv"], np.float32)
    b_qkv = np.asarray(inputs["b_qkv"], np.float32)
    W_proj = np.asarray(inputs["W_proj"], np.float32)
    b_proj = np.asarray(inputs["b_proj"], np.float32)

    in_maps = _shard_inputs(x, rope_cos, rope_sin, W_qkv, b_qkv, W_proj)
    outs = _run_spmd(in_maps)
    out = np.empty((B, N, EMBED), np.float32)
    for b in range(B):
        out[b] = outs[2 * b]["y"] + outs[2 * b + 1]["y"] + b_proj
    return out
